# revision 7
# baseline (speedup 1.0000x reference)
"""Trainium2 Bass kernel for nn_AdaptiveValuesMetadataAttention.

Shapes (hardcoded from the problem spec):
  values   [1, 8, 512, 256]  metadata [1, 8, 512, 64]
  w_meta_outer [64, 512]  w_qkv [256, 768]  w_meta_inner [64, 512]
  w_out [256, 256]  b_out [256]

Strategy: the outer source-level metadata attention selects, per source s,
the top-3 source windows (the +2*I diagonal boost guarantees slot 0 == s).
That top-k and the window gather are data-dependent *sharding* and run on
the host.  Each of the 8 NeuronCores then computes one source's inner
fused attention (queries = window slot-0 tokens, keys/values = all 3*512
window tokens) entirely on-device.

Per-core device kernel (all matmuls in float32r: fp32 storage, ~fp22
multiply precision at full PE rate):
  phase 1: project Qp/Qm/Kp/Km (transposed layouts) and V (token-major,
           with a ones-column per head for the softmax denominators)
  phase 2: per head pair (disjoint PE row/col groups for overlap):
           scoresT = Kp·QpT + Km·QmT  -> exp (scalar engine, fused
           1/sqrt(dh) scale) -> attn@V accumulation (ones-column yields
           the softmax sums in an extra psum row)
  phase 3: per-head normalize (broadcast reciprocal sums via a K=1
           matmul), then the output projection + bias, emitted transposed
           [dv, n]; the host transposes back and stacks cores.
"""

import numpy as np

B, S, N, DV, DM = 1, 8, 512, 256, 64
INNER, H, WS = 256, 8, 3
DH = INNER // H          # 32
W = WS * N               # 1536 kv tokens per window
SCALE = DH ** -0.5

_CACHE = {}


def _host_top_idx(values, metadata, w_meta_outer):
    meta_mean = metadata.mean(axis=2)                        # [B,S,DM]
    qk = meta_mean @ w_meta_outer                            # [B,S,2*INNER]
    qm = np.clip(qk[..., :INNER], -5, 5)
    km = np.clip(qk[..., INNER:], -5, 5)
    dots = np.einsum('bqd,bkd->bqk', qm, km) * (INNER ** -0.5)
    m = dots.max(-1, keepdims=True)
    e = np.exp(dots - m)
    attn = e / e.sum(-1, keepdims=True)
    attn = attn + 2.0 * np.eye(S, dtype=attn.dtype)
    # jax.lax.top_k: k largest, ties broken by lower index (stable)
    return np.argsort(-attn, axis=-1, kind='stable')[..., :WS]  # [B,S,WS]


def _build_bass():
    import concourse.bass as bass  # noqa: F401
    import concourse.tile as tile
    from concourse import bacc, mybir

    F32 = mybir.dt.float32
    F32R = mybir.dt.float32r
    EXP = mybir.ActivationFunctionType.Exp
    MIN = mybir.AluOpType.min
    MAX = mybir.AluOpType.max

    nc = bacc.Bacc(None, target_bir_lowering=False)

    kvT = nc.dram_tensor("kvT", [DV, W], F32R, kind="ExternalInput")
    kvmT = nc.dram_tensor("kvmT", [DM, W], F32R, kind="ExternalInput")
    wq = nc.dram_tensor("wq", [DV, INNER], F32R, kind="ExternalInput")
    wk = nc.dram_tensor("wk", [DV, INNER], F32R, kind="ExternalInput")
    wv = nc.dram_tensor("wv", [DV, INNER], F32R, kind="ExternalInput")
    wmq = nc.dram_tensor("wmq", [DM, INNER], F32R, kind="ExternalInput")
    wmk = nc.dram_tensor("wmk", [DM, INNER], F32R, kind="ExternalInput")
    wo = nc.dram_tensor("wo", [INNER, DV], F32R, kind="ExternalInput")
    ones1 = nc.dram_tensor("ones1", [1, 512], F32R, kind="ExternalInput")
    bo = nc.dram_tensor("bo", [DV, 1], F32, kind="ExternalInput")
    out = nc.dram_tensor("out", [DV, N], F32, kind="ExternalOutput")

    # head pairs scheduled together; row groups 32*(a%4) vs 32*(b%4)
    # differ by 64 so their PE row groups are disjoint.
    PAIRS = [(0, 2), (1, 3), (4, 6), (5, 7)]

    with tile.TileContext(nc) as tc:
        with (
            tc.tile_pool(name="w", bufs=1) as wp,
            tc.tile_pool(name="big", bufs=1) as bigp,
            tc.tile_pool(name="expp", bufs=3) as expp,
            tc.tile_pool(name="tails", bufs=2) as tailsb,
        ):
            # ---- persistent SBUF: inputs + weights --------------------
            kvT_sb = [wp.tile([128, W], F32R, tag=f"kvT{d}", name=f"kvT{d}") for d in range(2)]
            for d in range(2):
                nc.sync.dma_start(out=kvT_sb[d][:], in_=kvT[128 * d:128 * (d + 1), :])
            kvmT_sb = wp.tile([DM, W], F32R, tag="kvmT")
            nc.sync.dma_start(out=kvmT_sb[:], in_=kvmT[:])
            wq_sb = [wp.tile([128, INNER], F32R, tag=f"wq{d}", name=f"wq{d}") for d in range(2)]
            wk_sb = [wp.tile([128, INNER], F32R, tag=f"wk{d}", name=f"wk{d}") for d in range(2)]
            wv_sb = [wp.tile([128, INNER], F32R, tag=f"wv{d}", name=f"wv{d}") for d in range(2)]
            for d in range(2):
                nc.sync.dma_start(out=wq_sb[d][:], in_=wq[128 * d:128 * (d + 1), :])
                nc.sync.dma_start(out=wk_sb[d][:], in_=wk[128 * d:128 * (d + 1), :])
                nc.sync.dma_start(out=wv_sb[d][:], in_=wv[128 * d:128 * (d + 1), :])
            wmq_sb = wp.tile([DM, INNER], F32R, tag="wmq")
            wmk_sb = wp.tile([DM, INNER], F32R, tag="wmk")
            nc.sync.dma_start(out=wmq_sb[:], in_=wmq[:])
            nc.sync.dma_start(out=wmk_sb[:], in_=wmk[:])
            # wo split per head: woh[h] = wo[32h:32h+32, :] at partition base 0
            wo_sb = []
            for h in range(H):
                t = wp.tile([32, DV], F32R, tag=f"wo{h}", name=f"wo{h}")
                nc.sync.dma_start(out=t[:], in_=wo[32 * h:32 * h + 32, :])
                wo_sb.append(t)

            def bcast(src_ap, ap):
                return bass.AP(tensor=src_ap.tensor, offset=src_ap.offset, ap=ap)
            b_sb = wp.tile([128, 2], F32, tag="b")
            nc.sync.dma_start(out=b_sb[:, 0:1], in_=bo[0:128, :])
            nc.sync.dma_start(out=b_sb[:, 1:2], in_=bo[128:256, :])


            # ---- persistent SBUF: projection outputs ------------------
            QpT_sb = [bigp.tile([128, N], F32R, tag=f"QpT{t}", name=f"QpT{t}") for t in range(2)]
            QmT_sb = [bigp.tile([128, N], F32R, tag=f"QmT{t}", name=f"QmT{t}") for t in range(2)]
            KpT_sb = [bigp.tile([128, W], F32R, tag=f"KpT{t}", name=f"KpT{t}") for t in range(2)]
            KmT_sb = [bigp.tile([128, W], F32R, tag=f"KmT{t}", name=f"KmT{t}") for t in range(2)]
            V_sb = [bigp.tile([128, 33 * H], F32R, tag=f"V{c}", name=f"V{c}") for c in range(12)]
            OTn_sb = [bigp.tile([32, N], F32R, tag=f"OTn{h}", name=f"OTn{h}") for h in range(H)]

            def clip_copy(dst, src):
                nc.vector.tensor_scalar(dst, src, 5.0, -5.0, MIN, MAX)

            # ---- phase 1: projections ---------------------------------
            with tc.tile_pool(name="proj", bufs=4, space="PSUM") as projp:
                for t in range(2):
                    sl = slice(128 * t, 128 * (t + 1))
                    ps = projp.tile([128, N], F32, tag="proj")
                    nc.tensor.matmul(ps[:], wq_sb[0][:, sl], kvT_sb[0][:, 0:N],
                                     start=True, stop=False)
                    nc.tensor.matmul(ps[:], wq_sb[1][:, sl], kvT_sb[1][:, 0:N],
                                     start=False, stop=True)
                    clip_copy(QpT_sb[t][:], ps[:])
                    ps = projp.tile([128, N], F32, tag="proj")
                    nc.tensor.matmul(ps[:], wmq_sb[:, sl], kvmT_sb[:, 0:N])
                    clip_copy(QmT_sb[t][:], ps[:])
                    for bk in range(3):
                        fs = slice(512 * bk, 512 * (bk + 1))
                        ps = projp.tile([128, N], F32, tag="proj")
                        nc.tensor.matmul(ps[:], wk_sb[0][:, sl], kvT_sb[0][:, fs],
                                         start=True, stop=False)
                        nc.tensor.matmul(ps[:], wk_sb[1][:, sl], kvT_sb[1][:, fs],
                                         start=False, stop=True)
                        clip_copy(KpT_sb[t][:, fs], ps[:])
                        ps = projp.tile([128, N], F32, tag="proj")
                        nc.tensor.matmul(ps[:], wmk_sb[:, sl], kvmT_sb[:, fs])
                        clip_copy(KmT_sb[t][:, fs], ps[:])
                for c in range(12):
                    cs = slice(128 * c, 128 * (c + 1))
                    ps = projp.tile([128, DV], F32, tag="proj")
                    nc.tensor.matmul(ps[:], kvT_sb[0][:, cs], wv_sb[0][:],
                                     start=True, stop=False)
                    nc.tensor.matmul(ps[:], kvT_sb[1][:, cs], wv_sb[1][:],
                                     start=False, stop=True)
                    v3 = V_sb[c][:].rearrange("p (h w) -> p h w", w=33)
                    nc.vector.tensor_copy(
                        v3[:, :, 0:32],
                        ps[:].rearrange("p (h w) -> p h w", w=32))
                    nc.sync.dma_start(
                        out=v3[:, :, 32:33],
                        in_=bcast(ones1[0:1, :], [[0, 128], [1, 8], [1, 1]]))

            # ---- phase 2: attention per head pair ---------------------
            with (
                tc.tile_pool(name="sc", bufs=2, space="PSUM") as scp,
                tc.tile_pool(name="tail", bufs=2, space="PSUM") as tailp,
                tc.tile_pool(name="drp", bufs=2, space="DRAM") as drp,
            ):
                for pi, (a, b) in enumerate(PAIRS):
                    ga, gb = 32 * (a % 4), 32 * (b % 4)
                    ta, tb = a // 4, b // 4
                    outpsA = tailp.tile([33, N], F32, tag="outps", name="outpsA")
                    outpsB = tailp.tile([33, N], F32, tag="outps", name="outpsB")
                    for blk in range(4):
                        psA = scp.tile([128, 1536], F32, tag="sc", name="psA")
                        psB = scp.tile([128, 1536], F32, tag="sc", name="psB")
                        for j in range(3):
                            c = 3 * blk + j
                            cs = slice(128 * c, 128 * (c + 1))
                            js = slice(512 * j, 512 * (j + 1))
                            nc.tensor.matmul(
                                psA[:, js], KpT_sb[ta][ga:ga + 32, cs],
                                QpT_sb[ta][ga:ga + 32, :], start=True, stop=False,
                                tile_position=(ga, 0))
                            nc.tensor.matmul(
                                psB[:, js], KpT_sb[tb][gb:gb + 32, cs],
                                QpT_sb[tb][gb:gb + 32, :], start=True, stop=False,
                                tile_position=(gb, 0))
                            nc.tensor.matmul(
                                psA[:, js], KmT_sb[ta][ga:ga + 32, cs],
                                QmT_sb[ta][ga:ga + 32, :], start=False, stop=True,
                                tile_position=(ga, 0))
                            nc.tensor.matmul(
                                psB[:, js], KmT_sb[tb][gb:gb + 32, cs],
                                QmT_sb[tb][gb:gb + 32, :], start=False, stop=True,
                                tile_position=(gb, 0))
                        eA = expp.tile([128, 1536], F32R, tag="exp", name="eA")
                        eB = expp.tile([128, 1536], F32R, tag="exp", name="eB")
                        nc.scalar.activation(eA[:], psA[:], EXP, scale=SCALE)
                        nc.scalar.activation(eB[:], psB[:], EXP, scale=SCALE)
                        for j in range(3):
                            c = 3 * blk + j
                            js = slice(512 * j, 512 * (j + 1))
                            nc.tensor.matmul(
                                outpsA[0:33, :], V_sb[c][:, 33 * a:33 * a + 33],
                                eA[:, js], start=(c == 0), stop=(c == 11))
                            nc.tensor.matmul(
                                outpsB[0:33, :], V_sb[c][:, 33 * b:33 * b + 33],
                                eB[:, js], start=(c == 0), stop=(c == 11))
                    # tail: normalize each head of the pair
                    for h, outps in ((a, outpsA), (b, outpsB)):
                        stg = tailsb.tile([64, N], F32, tag="stg", name="stg")
                        nc.vector.tensor_copy(stg[0:33, :], outps[0:33, :])
                        sums_bc = tailsb.tile([32, N], F32, tag="sumbc",
                                              name="sums_bc")
                        sums_dr = drp.tile([1, N], F32, tag="sumdr",
                                           name="sums_dr")
                        nc.sync.dma_start(out=sums_dr[:], in_=stg[32:33, :])
                        nc.sync.dma_start(
                            out=sums_bc[:],
                            in_=bcast(sums_dr[0:1, :], [[0, 32], [1, N]]))
                        rcp = tailsb.tile([32, N], F32, tag="rcp", name="rcp")
                        nc.vector.reciprocal_approx_fast(out=rcp[:],
                                                         in_=sums_bc[:])
                        nc.vector.tensor_mul(OTn_sb[h][:], stg[0:32, :], rcp[:])

            # ---- phase 3: output projection + bias --------------------
            with tc.tile_pool(name="fin", bufs=2, space="PSUM") as finp:
                for d in range(2):
                    sl = slice(128 * d, 128 * (d + 1))
                    ops = finp.tile([128, N], F32, tag="fin", name="ops")
                    for h in range(H):
                        nc.tensor.matmul(ops[:], wo_sb[h][:, sl], OTn_sb[h][:],
                                         start=(h == 0), stop=(h == H - 1))
                    fin = tailsb.tile([128, N], F32, tag="fin", name="fin")
                    nc.vector.tensor_scalar_add(fin[:], ops[:], b_sb[:, d:d + 1])
                    nc.sync.dma_start(out=out[sl, :], in_=fin[:])

    nc.compile()
    return nc


def _get_nc():
    if "nc" not in _CACHE:
        _CACHE["nc"] = _build_bass()
    return _CACHE["nc"]


def build_in_maps(values, metadata, w_qkv, w_meta_inner, w_out, b_out, top_idx):
    f = np.float32
    wq = np.ascontiguousarray(w_qkv[:, :INNER], dtype=f)
    wk = np.ascontiguousarray(w_qkv[:, INNER:2 * INNER], dtype=f)
    wv = np.ascontiguousarray(w_qkv[:, 2 * INNER:], dtype=f)
    wmq = np.ascontiguousarray(w_meta_inner[:, :INNER], dtype=f)
    wmk = np.ascontiguousarray(w_meta_inner[:, INNER:], dtype=f)
    wo = np.ascontiguousarray(w_out, dtype=f)
    ones1 = np.ones((1, 512), dtype=f)
    bo = np.ascontiguousarray(b_out.reshape(DV, 1), dtype=f)
    in_maps = []
    for s in range(S):
        idx = top_idx[0, s]
        kvT = np.ascontiguousarray(
            values[0, idx].reshape(W, DV).T, dtype=f)
        kvmT = np.ascontiguousarray(
            metadata[0, idx].reshape(W, DM).T, dtype=f)
        in_maps.append({
            "kvT": kvT, "kvmT": kvmT, "wq": wq, "wk": wk, "wv": wv,
            "wmq": wmq, "wmk": wmk, "wo": wo, "bo": bo, "ones1": ones1,
        })
    return in_maps


def kernel(values, metadata, w_meta_outer, w_qkv, w_meta_inner, w_out, b_out,
           _trace=False):
    from concourse.bass_utils import run_bass_kernel_spmd

    values = np.asarray(values, dtype=np.float32)
    metadata = np.asarray(metadata, dtype=np.float32)
    w_meta_outer = np.asarray(w_meta_outer, dtype=np.float32)
    w_qkv = np.asarray(w_qkv, dtype=np.float32)
    w_meta_inner = np.asarray(w_meta_inner, dtype=np.float32)
    w_out = np.asarray(w_out, dtype=np.float32)
    b_out = np.asarray(b_out, dtype=np.float32)

    top_idx = _host_top_idx(values, metadata, w_meta_outer)
    assert (top_idx[0, :, 0] == np.arange(S)).all(), top_idx

    in_maps = build_in_maps(values, metadata, w_qkv, w_meta_inner, w_out,
                            b_out, top_idx)
    nc = _get_nc()
    res = run_bass_kernel_spmd(nc, in_maps, core_ids=list(range(S)),
                               trace=_trace)
    out = np.stack([res.results[s]["out"].T for s in range(S)], axis=0)
    _CACHE["last_result"] = res
    return out.reshape(B, S, N, DV)
